# revision 17
# baseline (speedup 1.0000x reference)
"""HGCL forward on 8 Trainium2 NeuronCores.

Strategy: the memory-bound core of this model is 8 SpMMs over ~10M directed
edges (2 GNN layers x 3 graphs + 2 meta aggregations). Each SpMM runs on
device, dest-node-sharded across the 8 cores: edge messages are gathered from
HBM feature tables with dma_gather (256B descriptors carrying bf16 feature
PAIRS bit-cast as f32, round-robined over all 4 SWDGE queues), reduced into
64-row destination blocks with per-chunk one-hot matmuls in bf16 on the
tensor engine (PSUM-accumulated per superblock), and written back dense.

Descriptor count is the bottleneck (~2.26 ns/descriptor at 4 queues), so
bins are packed back-to-back with NO per-bin chunk padding: a 128-slot chunk
may span several (dest-block, parity) bins, and each (chunk, bin) segment
gets its own matmul pass whose per-pass weight column zeroes foreign slots.
One-hot selectors are built in bulk on the vector engine (two bf16
scalar_tensor_tensor ops per job using stride-0 broadcast access patterns)
from rmb/w pass tables resident in SBUF. Cheap dense glue (gating, l2-norms,
means, meta MLPs, softmax/einsum head) runs on host between the three device
launches.
"""
import numpy as np, sys
sys.path.insert(0, '/opt/trn_rl_repo')
import ml_dtypes
import concourse.bacc as bacc
import concourse.tile as tile
import concourse.mybir as mybir
from concourse import bass_utils

USER_N, ITEM_N, D, K = 50000, 80000, 64, 4
N = USER_N + ITEM_N
NC = 8
US, IS = USER_N // NC, ITEM_N // NC   # 6250, 10000 per-core shards
RANGE = 32768        # pair-rows per gather table (= 65536 nodes)
P = 128              # slots per chunk (PE contraction width)
B = 64               # dest rows per block (one-hot width)
SBLK = 8             # blocks of B per superblock (psum tile = [64, SBLK*64])
SUB = 1024           # idxs per dma_gather instruction (HW ucode max)
NQ = 4               # SWDGE queues (ucode MAX_SWDGE_QUEUES)
TCH = 32             # chunks per pipeline job
EPS = 1e-12
BF16 = ml_dtypes.bfloat16

# ---------------- host planning ----------------

def plan_graph(rows_l, cols_l, ws_l, n_dest_local, n_src):
    """Bin edges by (superblock S, source range r, dest block b, src parity)
    per core. Within each (S, r) group, bins are packed back-to-back (slot
    count per bin = max count across cores, NO rounding); chunks are 128-slot
    windows over the packed group, and every (chunk, bin) overlap becomes one
    matmul pass. All cores run an identical program."""
    nblocks = -(-n_dest_local // B)
    nS = -(-nblocks // SBLK)
    nR = -(-(n_src // 2) // RANGE)
    counts = np.zeros((NC, nR, nblocks, 2), dtype=np.int64)
    for c in range(NC):
        b = rows_l[c] // B
        r = (cols_l[c] >> 1) // RANGE
        par = cols_l[c] & 1
        np.add.at(counts, (c, r, b, par), 1)
    maxc = counts.max(axis=0)                   # [nR, nblocks, 2]

    groups = []          # per (S, r): slot/chunk/pass layout
    bin_slot = np.full((nR, nblocks, 2), -1, dtype=np.int64)
    pass_of = {}         # (global_chunk, b, par) -> global pass index
    pass_meta = []       # per pass: (S, b, par, chunk_global)
    slot_off = 0
    chunk_off = 0
    npass = 0
    for S in range(nS):
        for r in range(nR):
            bins = []
            g_slot0 = slot_off
            for b in range(S * SBLK, min((S + 1) * SBLK, nblocks)):
                for par in (0, 1):
                    n = int(maxc[r, b, par])
                    if n == 0:
                        continue
                    bin_slot[r, b, par] = slot_off
                    bins.append((b, par, slot_off, n))
                    slot_off += n
            if not bins:
                continue
            g_slots = slot_off - g_slot0
            nch = -(-g_slots // P)
            slot_off = g_slot0 + nch * P        # pad group tail to whole chunks
            # jobs: runs of TCH chunks
            jobs = []
            for j0 in range(0, nch, TCH):
                j1 = min(j0 + TCH, nch)
                jp0 = npass
                chunk_passes = []               # per chunk: [(b, par, pass)]
                for ci in range(j0, j1):
                    c_lo = g_slot0 + ci * P
                    c_hi = c_lo + P
                    segs = []
                    for b, par, s0, n in bins:
                        if s0 < c_hi and s0 + n > c_lo:
                            pass_of[(chunk_off + ci, b, par)] = npass
                            pass_meta.append((S, b, par, chunk_off + ci))
                            segs.append((b, par, npass))
                            npass += 1
                    chunk_passes.append(segs)
                jobs.append(dict(c0=j0, c1=j1, p0=jp0, p1=npass,
                                 chunk_passes=chunk_passes))
            groups.append(dict(S=S, r=r, slot0=g_slot0, nch=nch,
                               chunk0=chunk_off, jobs=jobs))
            chunk_off += nch
    total_slots = slot_off
    total_chunks = chunk_off
    # first/last pass per (S, b) for psum start/stop + written-width per S
    first_ps, last_ps = {}, {}
    nbw = {}
    for pi, (S, b, par, cg) in enumerate(pass_meta):
        if (S, b) not in first_ps:
            first_ps[(S, b)] = pi
        last_ps[(S, b)] = pi
        nbw[S] = max(nbw.get(S, 0), b - S * SBLK + 1)
    plan = dict(nblocks=nblocks, nS=nS, nR=nR, groups=groups,
                total_slots=total_slots, total_chunks=total_chunks,
                npass=npass, first_ps=first_ps, last_ps=last_ps, nbw=nbw,
                n_src=n_src)

    # sorted pass-key table for vectorized (chunk, b, par) -> pass lookup
    pk = np.array([(cg * nblocks + b) * 2 + par
                   for (S, b, par, cg) in pass_meta], dtype=np.int64)
    pk_order = np.argsort(pk, kind='stable')
    pk_sorted = pk[pk_order]

    # ---- per-core data: idx per slot, rmb/w per (pass, slotpos) ----
    percore = []
    for c in range(NC):
        rs, cs_, ws_ = rows_l[c], cols_l[c], ws_l[c]
        b_s = rs // B
        r_s = (cs_ >> 1) // RANGE
        par_s = (cs_ & 1).astype(np.int64)
        # position within bin
        key = (r_s.astype(np.int64) * nblocks + b_s) * 2 + par_s
        so = np.argsort(key, kind='stable')
        pos = np.zeros(len(rs), dtype=np.int64)
        _, fi, ct = np.unique(key[so], return_index=True, return_counts=True)
        for f0, c0 in zip(fi, ct):
            pos[so[f0:f0 + c0]] = np.arange(c0)
        base = bin_slot[r_s, b_s, par_s]
        slot = base + pos
        idx_flat = np.zeros(total_slots, dtype=np.int16)
        idx_flat[slot] = ((cs_ >> 1) % RANGE).astype(np.int16)
        cg = slot // P
        sp = slot % P
        ek = (cg * nblocks + b_s) * 2 + par_s
        pidx = pk_order[np.searchsorted(pk_sorted, ek)]
        rmb_arr = np.zeros((npass, P), dtype=BF16)
        w_arr = np.zeros((npass, P), dtype=BF16)
        rmb_arr[pidx, sp] = (rs - b_s * B).astype(BF16)
        w_arr[pidx, sp] = ws_.astype(BF16)
        idx2d = np.tile(idx_flat.reshape(-1, 16).T, (8, 1))
        percore.append(dict(idx=np.ascontiguousarray(idx2d),
                            rmb=np.ascontiguousarray(rmb_arr.T),
                            w=np.ascontiguousarray(w_arr.T)))
    return plan, percore


def build_spmm_graph(nc, pools, name, plan, iota_b, qctr):
    f32 = mybir.dt.float32
    bf16 = mybir.dt.bfloat16
    n_pair = plan['n_src'] // 2
    tabs = [nc.dram_tensor(f"{name}_tab{r}", [min(RANGE, n_pair - r * RANGE), 64],
                           f32, kind="ExternalInput")
            for r in range(plan['nR'])]
    idx_d = nc.dram_tensor(f"{name}_idx", [P, plan['total_slots'] // 16],
                           mybir.dt.int16, kind="ExternalInput")
    rmb_d = nc.dram_tensor(f"{name}_rmb", [P, plan['npass']], bf16,
                           kind="ExternalInput")
    w_d = nc.dram_tensor(f"{name}_w", [P, plan['npass']], bf16,
                         kind="ExternalInput")
    out_d = nc.dram_tensor(f"{name}_out", [plan['nblocks'] * B, 64], f32,
                           kind="ExternalOutput")
    sbuf, psum, gpool, selpool, eqpool, resid = pools
    # rmb/w pass tables stay resident in SBUF for the whole launch
    rmb_t = resid.tile([P, plan['npass']], bf16, tag=f"rmb_{name}")
    w_t = resid.tile([P, plan['npass']], bf16, tag=f"w_{name}")
    nc.sync.dma_start(rmb_t[:], rmb_d[:])
    nc.sync.dma_start(w_t[:], w_d[:])
    first_ps, last_ps = plan['first_ps'], plan['last_ps']
    from collections import defaultdict
    byS = defaultdict(list)
    for g in plan['groups']:
        byS[g['S']].append(g)
    for S, glist in sorted(byS.items()):
        # one full PSUM bank per dest block: accumulation chains of different
        # blocks never share a 2KB zero region, so matmuls can be emitted
        # chunk-major per job with no superblock barrier (start=True wipes
        # only its own bank)
        pts = {}

        def pt_of(b):
            bi = b % SBLK
            if bi not in pts:
                pts[bi] = psum.tile([B, 512], mybir.dt.float32,
                                    tag=f"ps{bi}", name=f"pt{bi}")
            return pts[bi]

        for g in glist:
            for job in g['jobs']:
                c0, c1 = job['c0'], job['c1']
                nch = c1 - c0
                np_j = job['p1'] - job['p0']
                s_lo = g['slot0'] + c0 * P
                s_hi = g['slot0'] + c1 * P
                it = gpool.tile([P, (s_hi - s_lo) // 16], mybir.dt.int16,
                                tag="idx")
                nc.sync.dma_start(it[:], idx_d[:, s_lo // 16:s_hi // 16])
                gb = gpool.tile([P, nch * 64], f32, tag="gbuf")
                gb3 = gb[:].rearrange("p (c f) -> p c f", f=64)
                gbb = gb[:].bitcast(bf16).rearrange("p (c f) -> p c f", f=128)
                for s0 in range(0, s_hi - s_lo, SUB):
                    gsub = min(SUB, s_hi - s_lo - s0)
                    nc.gpsimd.dma_gather(
                        out_ap=gb3[:, s0 // P:(s0 + gsub) // P, :],
                        in_ap=tabs[g['r']][:],
                        idxs_ap=it[:, s0 // 16:(s0 + gsub) // 16],
                        num_idxs=gsub, num_idxs_reg=gsub, elem_size=64,
                        queue_num=qctr[0] % NQ)
                    qctr[0] += 1
                # bulk selector build over this job's passes:
                # sel[p, k, d] = (rmb[p, p0+k] == d) * w[p, p0+k]
                eq = eqpool.tile([P, np_j * B], bf16, tag="eq")
                eq3 = eq[:].rearrange("p (c d) -> p c d", d=B)
                sel = selpool.tile([P, np_j * B], bf16, tag="sel")
                sel3 = sel[:].rearrange("p (c d) -> p c d", d=B)
                rt_b = rmb_t[:, job['p0']:job['p1']].rearrange(
                    "p (c u) -> p c u", u=1).broadcast_to([P, np_j, B])
                wt_b = w_t[:, job['p0']:job['p1']].rearrange(
                    "p (c u) -> p c u", u=1).broadcast_to([P, np_j, B])
                io_b = iota_b[:].rearrange("p (u d) -> p u d", u=1).broadcast_to(
                    [P, np_j, B])
                nc.vector.scalar_tensor_tensor(
                    out=eq3, in0=rt_b, scalar=1.0, in1=io_b,
                    op0=mybir.AluOpType.mult, op1=mybir.AluOpType.is_equal)
                nc.vector.scalar_tensor_tensor(
                    out=sel3, in0=eq3, scalar=1.0, in1=wt_b,
                    op0=mybir.AluOpType.mult, op1=mybir.AluOpType.mult)
                for ci_l, segs in enumerate(job['chunk_passes']):
                    for b, par, pi in segs:
                        pt = pt_of(b)
                        k = pi - job['p0']
                        nc.tensor.matmul(
                            pt[:, :64],
                            lhsT=sel[:, k * B:(k + 1) * B],
                            rhs=gbb[:, ci_l, par * 64:(par + 1) * 64],
                            start=(pi == first_ps[(S, b)]),
                            stop=(pi == last_ps[(S, b)]))
                        if pi == last_ps[(S, b)]:
                            # this block's chain just closed: drain its bank
                            acc = sbuf.tile([B, 64], mybir.dt.float32,
                                            tag="acc")
                            nc.scalar.activation(
                                out=acc[:], in_=pt[:, :64],
                                func=mybir.ActivationFunctionType.Copy)
                            nc.sync.dma_start(out_d[b * B:(b + 1) * B, :],
                                              acc[:])


def build_neff(plans):
    nc = bacc.Bacc("TRN2", target_bir_lowering=False, debug=False,
                   num_devices=NC, num_swdge_queues=NQ)
    with tile.TileContext(nc) as tc:
        with tc.tile_pool(name="sbuf", bufs=3) as sbuf, \
             tc.tile_pool(name="gpool", bufs=8) as gpool, \
             tc.tile_pool(name="selpool", bufs=8) as selpool, \
             tc.tile_pool(name="eqpool", bufs=3) as eqpool, \
             tc.tile_pool(name="resid", bufs=1) as resid, \
             tc.tile_pool(name="psum", bufs=1, space="PSUM") as psum, \
             tc.tile_pool(name="const", bufs=1) as constp:
            iota_i = constp.tile([P, B], mybir.dt.int32)
            nc.gpsimd.iota(iota_i[:], pattern=[[1, B]], base=0, channel_multiplier=0)
            iota_b = constp.tile([P, B], mybir.dt.bfloat16)
            nc.vector.tensor_copy(out=iota_b[:], in_=iota_i[:])
            qctr = [0]
            for name, plan in plans.items():
                build_spmm_graph(nc, (sbuf, psum, gpool, selpool, eqpool, resid),
                                 name, plan, iota_b, qctr)
    nc.compile()
    return nc


def to_pair_table(feat):
    """f32 [n, 64] -> bf16 pair rows bit-cast to f32 [n/2, 64]."""
    xb = feat.astype(BF16).reshape(-1, 128).view(np.uint16)
    return np.ascontiguousarray(xb).view(np.float32)


def split_tab(feat):
    pt = to_pair_table(feat)
    return [np.ascontiguousarray(pt[r * RANGE:min((r + 1) * RANGE, len(pt))])
            for r in range(-(-len(pt) // RANGE))]


def run_launch(nc, plans, percores, tables):
    tabs = {name: split_tab(tables[name]) for name in plans}
    in_maps = []
    for c in range(NC):
        m = {}
        for name in plans:
            pc = percores[name][c]
            m[f"{name}_idx"] = pc['idx']
            m[f"{name}_rmb"] = pc['rmb']
            m[f"{name}_w"] = pc['w']
            for r, t in enumerate(tabs[name]):
                m[f"{name}_tab{r}"] = t
        in_maps.append(m)
    import os
    trace = os.environ.get('KTRACE', '0') == '1'
    res = bass_utils.run_bass_kernel_spmd(nc, in_maps, core_ids=list(range(NC)),
                                          trace=trace)
    if res.exec_time_ns:
        globals()['HW_NS'] = globals().get('HW_NS', 0) + int(res.exec_time_ns)
    outs = {}
    for name, plan in plans.items():
        outs[name] = [res.results[c][f"{name}_out"] for c in range(NC)]
    return outs


def asm_users(parts):
    return np.concatenate([p[:US] for p in parts], 0)

def asm_items(parts):
    return np.concatenate([p[:IS] for p in parts], 0)

def asm_ui(parts):
    u = np.concatenate([p[:US] for p in parts], 0)
    i = np.concatenate([p[US:US + IS] for p in parts], 0)
    return np.concatenate([u, i], 0)

# ---------------- host glue (numpy port of reference) ----------------

def l2n(x):
    return x / np.maximum(np.linalg.norm(x, axis=-1, keepdims=True), EPS)

def mlp_np(x, Wp, bp, Wo, bo):
    h = x @ Wp + bp
    h = np.where(h > 0, h, 0.25 * h).astype(np.float32)
    return l2n(h @ Wo + bo)

def norm_w(row, col, val, n):
    deg = np.bincount(row, weights=val, minlength=n).astype(np.float32)
    dis = np.where(deg > 0, np.where(deg > 0, deg, 1.0) ** -0.5, 0.0).astype(np.float32)
    return (val * dis[row] * dis[col]).astype(np.float32)

_CACHE = {}

def _shard_users(r):
    return r // US, r % US

def _shard_items(r):
    return r // IS, r % IS

def _shard_ui(r):
    isu = r < USER_N
    c = np.where(isu, r // US, (r - USER_N) // IS)
    loc = np.where(isu, r % US, US + (r - USER_N) % IS)
    return c, loc

def _split(rows, cols, ws, shard_fn):
    c, loc = shard_fn(rows)
    out = ([], [], [])
    for cc in range(NC):
        m = c == cc
        out[0].append(loc[m])
        out[1].append(cols[m])
        out[2].append(ws[m])
    return out


def kernel(**inp):
    g = lambda k: np.asarray(inp[k])
    uu_row, uu_col, uu_val = g('uu_row'), g('uu_col'), g('uu_val')
    ii_row, ii_col, ii_val = g('ii_row'), g('ii_col'), g('ii_val')
    ui_u, ui_i, ui_val = g('ui_u'), g('ui_i'), g('ui_val')
    user_emb, item_emb = g('user_emb'), g('item_emb')

    # symmetric ui adjacency
    ui_row = np.concatenate([ui_u, ui_i + USER_N])
    ui_colS = np.concatenate([ui_i + USER_N, ui_u])
    ui_v2 = np.concatenate([ui_val, ui_val])

    w_uu = norm_w(uu_row, uu_col, uu_val, USER_N)
    w_ii = norm_w(ii_row, ii_col, ii_val, ITEM_N)
    w_ui = norm_w(ui_row, ui_colS, ui_v2, N)

    import hashlib
    ck = hashlib.sha1(b''.join(
        a[::131].tobytes() for a in
        (uu_row, uu_col, ii_row, ii_col, ui_u, ui_i))).hexdigest()
    if _CACHE.get('key') != ck:
        _CACHE.clear()
        _CACHE['key'] = ck

    if 'A' not in _CACHE:
        pu, du = plan_graph(*_split(uu_row, uu_col, w_uu, _shard_users), US, USER_N)
        pi, di = plan_graph(*_split(ii_row, ii_col, w_ii, _shard_items), IS, ITEM_N)
        pui, dui = plan_graph(*_split(ui_row, ui_colS, w_ui, _shard_ui), US + IS, N)
        plansA = dict(uu=pu, ii=pi, ui=pui)
        dataA = dict(uu=du, ii=di, ui=dui)
        pmu, dmu = plan_graph(*_split(ui_u, ui_i, ui_val, _shard_users), US, ITEM_N)
        pmi, dmi = plan_graph(*_split(ui_i, ui_u, ui_val, _shard_items), IS, USER_N)
        plansB = dict(mu=pmu, mi=pmi)
        dataB = dict(mu=dmu, mi=dmi)
        _CACHE['A'] = (plansA, dataA, build_neff(plansA))
        _CACHE['B'] = (plansB, dataB, build_neff(plansB))
    plansA, dataA, ncA = _CACHE['A']
    plansB, dataB, ncB = _CACHE['B']

    # zero-degree dest rows: their psum region is never written on device,
    # so the copied-out values are garbage — mask them to the exact 0 the
    # reference's segment_sum produces.
    z_uu = np.bincount(uu_row, minlength=USER_N) == 0
    z_ii = np.bincount(ii_row, minlength=ITEM_N) == 0
    z_ui = np.bincount(ui_row, minlength=N) == 0
    z_mu = np.bincount(ui_u, minlength=USER_N) == 0
    z_mi = np.bincount(ui_i, minlength=ITEM_N) == 0

    # gate (host)
    uu0 = (user_emb * (1 / (1 + np.exp(-(user_emb @ g('gwu') + g('gwub')))))).astype(np.float32)
    ii0 = (item_emb * (1 / (1 + np.exp(-(item_emb @ g('gwi') + g('gwib')))))).astype(np.float32)
    uiE = np.concatenate([user_emb, item_emb], 0)
    all_u, all_i, all_ui = [uu0], [ii0], [uiE]
    uE, iE = uu0, ii0
    for _ in range(2):
        o = run_launch(ncA, plansA, dataA,
                       dict(uu=uE, ii=iE, ui=uiE))
        u0 = asm_users(o['uu'])
        i0 = asm_items(o['ii'])
        ui0 = asm_ui(o['ui'])
        u0[z_uu] = 0.0
        i0[z_ii] = 0.0
        ui0[z_ui] = 0.0
        uE = ((u0 + ui0[:USER_N]) * 0.5).astype(np.float32)
        iE = ((i0 + ui0[USER_N:]) * 0.5).astype(np.float32)
        uiE = np.concatenate([uE, iE], 0)
        all_u.append(l2n(u0).astype(np.float32))
        all_i.append(l2n(i0).astype(np.float32))
        all_ui.append(l2n(ui0).astype(np.float32))
    userEmb = np.mean(np.stack(all_u, 1), 1).astype(np.float32)
    itemEmb = np.mean(np.stack(all_i, 1), 1).astype(np.float32)
    uiEmb = np.mean(np.stack(all_ui, 1), 1).astype(np.float32)
    ui_uE, ui_iE = uiEmb[:USER_N], uiEmb[USER_N:]

    o = run_launch(ncB, plansB, dataB, dict(mu=ui_iE, mi=ui_uE))
    uneigh = asm_users(o['mu'])
    ineigh = asm_items(o['mi'])
    uneigh[z_mu] = 0.0
    ineigh[z_mi] = 0.0

    tu = (np.concatenate([userEmb, ui_uE, uneigh], 1) @ g('meta_u_W') + g('meta_u_b')).astype(np.float32)
    ti = (np.concatenate([itemEmb, ui_iE, ineigh], 1) @ g('meta_i_W') + g('meta_i_b')).astype(np.float32)
    mu1 = mlp_np(tu, g('m0_Wp'), g('m0_bp'), g('m0_Wo'), g('m0_bo')).reshape(-1, D, K)
    mu2 = mlp_np(tu, g('m1_Wp'), g('m1_bp'), g('m1_Wo'), g('m1_bo')).reshape(-1, K, D)
    mi1 = mlp_np(ti, g('m2_Wp'), g('m2_bp'), g('m2_Wo'), g('m2_bo')).reshape(-1, D, K)
    mi2 = mlp_np(ti, g('m3_Wp'), g('m3_bp'), g('m3_Wo'), g('m3_bo')).reshape(-1, K, D)

    def smax(x, ax):
        e = np.exp(x - x.max(axis=ax, keepdims=True))
        return (e / e.sum(axis=ax, keepdims=True)).astype(np.float32)
    lwu1 = smax(mu1 + mu1.mean(0), 1)
    lwu2 = smax(mu2 + mu2.mean(0), 1)
    lwi1 = smax(mi1 + mi1.mean(0), 1)
    lwi2 = smax(mi2 + mi2.mean(0), 1)
    tus = np.einsum('nd,ndk->nk', userEmb, lwu1)
    tus = np.einsum('nk,nkd->nd', tus, lwu2)
    tis = np.einsum('nd,ndk->nk', itemEmb, lwi1)
    tis = np.einsum('nk,nkd->nd', tis, lwi2)
    return np.concatenate([userEmb + tus, itemEmb + tis], 0).astype(np.float32)


# revision 24
# speedup vs baseline: 1.2698x; 1.2698x over previous
"""HGCL forward on 8 Trainium2 NeuronCores.

Strategy: the memory-bound core of this model is 8 SpMMs over ~10M directed
edges (2 GNN layers x 3 graphs + 2 meta aggregations). Each SpMM runs on
device, dest-node-sharded across the 8 cores: edge messages are gathered from
HBM feature tables with dma_gather (256B descriptors carrying bf16 feature
PAIRS bit-cast as f32, round-robined over all 4 SWDGE queues), reduced into
64-row destination blocks with per-chunk one-hot matmuls in bf16 on the
tensor engine (PSUM-accumulated per superblock), and written back dense.

Descriptor count is the bottleneck (~2.26 ns/descriptor at 4 queues), so
bins are packed back-to-back with NO per-bin chunk padding: a 128-slot chunk
may span several (dest-block, parity) bins, and each (chunk, bin) segment
gets its own matmul pass whose per-pass weight column zeroes foreign slots.
One-hot selectors are built in bulk on the vector engine (two bf16
scalar_tensor_tensor ops per job using stride-0 broadcast access patterns)
from rmb/w pass tables resident in SBUF. Cheap dense glue (gating, l2-norms,
means, meta MLPs, softmax/einsum head) runs on host between the three device
launches.
"""
import numpy as np, sys
sys.path.insert(0, '/opt/trn_rl_repo')
import ml_dtypes
import concourse.bacc as bacc
import concourse.tile as tile
import concourse.mybir as mybir
from concourse import bass_utils

USER_N, ITEM_N, D, K = 50000, 80000, 64, 4
N = USER_N + ITEM_N
NC = 8
US, IS = USER_N // NC, ITEM_N // NC   # 6250, 10000 per-core shards
RANGE = 32768        # pair-rows per gather table (= 65536 nodes)
P = 128              # slots per chunk (PE contraction width)
B = 64               # dest rows per block (one-hot width)
SBLK = 8             # blocks of B per superblock (psum tile = [64, SBLK*64])
SUB = 1024           # idxs per dma_gather instruction (HW ucode max)
NQ = 4               # SWDGE queues (ucode MAX_SWDGE_QUEUES)
TCH = 32             # chunks per pipeline job
EPS = 1e-12
BF16 = ml_dtypes.bfloat16

# ---------------- host planning ----------------

def plan_graph(rows_l, cols_l, ws_l, n_dest_local, n_src):
    """Bin edges by (superblock S, source range r, dest block b, src parity)
    per core. Within each (S, r) group, bins are packed back-to-back (slot
    count per bin = max count across cores, NO rounding); chunks are 128-slot
    windows over the packed group, and every (chunk, bin) overlap becomes one
    matmul pass. All cores run an identical program."""
    nblocks = -(-n_dest_local // B)
    nS = -(-nblocks // SBLK)
    nR = -(-(n_src // 2) // RANGE)
    nD = nR * 2

    # --- balance dest rows into blocks so every core's (r, par) bin counts
    # approach the same per-block quota (shrinks the max-across-cores pad) ---
    deg_all = np.zeros((NC, n_dest_local, nD), dtype=np.int32)
    for c in range(NC):
        dim = ((cols_l[c] >> 1) // RANGE).astype(np.int64) * 2 + (cols_l[c] & 1)
        np.add.at(deg_all, (c, rows_l[c].astype(np.int64), dim), 1)
    tgt = deg_all.sum(axis=(0, 1)) / (NC * nblocks)  # shared quota [nD]
    blockof_l, rank_l, pos_l = [], [], []
    for c in range(NC):
        deg = deg_all[c].astype(np.float64)
        order = np.argsort(-deg.sum(1), kind='stable')
        loads = np.zeros((nblocks, nD))
        over = np.zeros(nblocks)                 # current overflow sum
        nin = np.zeros(nblocks, dtype=np.int64)
        blockof = np.empty(n_dest_local, dtype=np.int64)
        rank = np.empty(n_dest_local, dtype=np.int64)
        for row in order:
            d = deg[row]
            inc = np.clip(loads + d - tgt, 0, None).sum(1) - over
            inc[nin >= B] = np.inf
            b = int(np.argmin(inc))
            blockof[row] = b
            rank[row] = nin[b]
            loads[b] += d
            over[b] += inc[b]
            nin[b] += 1
        blockof_l.append(blockof)
        rank_l.append(rank)
        pos_l.append(blockof * B + rank)

    counts = np.zeros((NC, nR, nblocks, 2), dtype=np.int64)
    for c in range(NC):
        b = blockof_l[c][rows_l[c]]
        r = (cols_l[c] >> 1) // RANGE
        par = cols_l[c] & 1
        np.add.at(counts, (c, r, b, par), 1)
    maxc = counts.max(axis=0)                   # [nR, nblocks, 2]

    groups = []          # per (S, r): slot/chunk/pass layout
    bin_slot = np.full((nR, nblocks, 2), -1, dtype=np.int64)
    pass_of = {}         # (global_chunk, b, par) -> global pass index
    pass_meta = []       # per pass: (S, b, par, chunk_global)
    slot_off = 0
    chunk_off = 0
    npass = 0
    for S in range(nS):
        for r in range(nR):
            bins = []
            g_slot0 = slot_off
            for b in range(S * SBLK, min((S + 1) * SBLK, nblocks)):
                for par in (0, 1):
                    n = int(maxc[r, b, par])
                    if n == 0:
                        continue
                    bin_slot[r, b, par] = slot_off
                    bins.append((b, par, slot_off, n))
                    slot_off += n
            if not bins:
                continue
            g_slots = slot_off - g_slot0
            nch = -(-g_slots // P)
            slot_off = g_slot0 + nch * P        # pad group tail to whole chunks
            # jobs: runs of TCH chunks
            jobs = []
            for j0 in range(0, nch, TCH):
                j1 = min(j0 + TCH, nch)
                jp0 = npass
                chunk_passes = []               # per chunk: [(b, par, pass)]
                for ci in range(j0, j1):
                    c_lo = g_slot0 + ci * P
                    c_hi = c_lo + P
                    segs = []
                    for b, par, s0, n in bins:
                        if s0 < c_hi and s0 + n > c_lo:
                            pass_of[(chunk_off + ci, b, par)] = npass
                            pass_meta.append((S, b, par, chunk_off + ci))
                            segs.append((b, par, npass))
                            npass += 1
                    chunk_passes.append(segs)
                jobs.append(dict(c0=j0, c1=j1, p0=jp0, p1=npass,
                                 chunk_passes=chunk_passes))
            groups.append(dict(S=S, r=r, slot0=g_slot0, nch=nch,
                               chunk0=chunk_off, jobs=jobs))
            chunk_off += nch
    total_slots = slot_off
    total_chunks = chunk_off
    # first/last pass per (S, b) for psum start/stop + written-width per S
    first_ps, last_ps = {}, {}
    nbw = {}
    for pi, (S, b, par, cg) in enumerate(pass_meta):
        if (S, b) not in first_ps:
            first_ps[(S, b)] = pi
        last_ps[(S, b)] = pi
        nbw[S] = max(nbw.get(S, 0), b - S * SBLK + 1)
    plan = dict(nblocks=nblocks, nS=nS, nR=nR, groups=groups,
                total_slots=total_slots, total_chunks=total_chunks,
                npass=npass, first_ps=first_ps, last_ps=last_ps, nbw=nbw,
                n_src=n_src)

    # sorted pass-key table for vectorized (chunk, b, par) -> pass lookup
    pk = np.array([(cg * nblocks + b) * 2 + par
                   for (S, b, par, cg) in pass_meta], dtype=np.int64)
    pk_order = np.argsort(pk, kind='stable')
    pk_sorted = pk[pk_order]

    # ---- per-core data: idx per slot, rmb/w per (pass, slotpos) ----
    percore = []
    for c in range(NC):
        rs, cs_, ws_ = rows_l[c], cols_l[c], ws_l[c]
        b_s = blockof_l[c][rs]
        rk_s = rank_l[c][rs]
        r_s = (cs_ >> 1) // RANGE
        par_s = (cs_ & 1).astype(np.int64)
        # position within bin
        key = (r_s.astype(np.int64) * nblocks + b_s) * 2 + par_s
        so = np.argsort(key, kind='stable')
        pos = np.zeros(len(rs), dtype=np.int64)
        _, fi, ct = np.unique(key[so], return_index=True, return_counts=True)
        for f0, c0 in zip(fi, ct):
            pos[so[f0:f0 + c0]] = np.arange(c0)
        base = bin_slot[r_s, b_s, par_s]
        slot = base + pos
        idx_flat = np.zeros(total_slots, dtype=np.int16)
        idx_flat[slot] = ((cs_ >> 1) % RANGE).astype(np.int16)
        cg = slot // P
        sp = slot % P
        ek = (cg * nblocks + b_s) * 2 + par_s
        pidx = pk_order[np.searchsorted(pk_sorted, ek)]
        rmb_arr = np.zeros((npass, P), dtype=BF16)
        w_arr = np.zeros((npass, P), dtype=BF16)
        rmb_arr[pidx, sp] = rk_s.astype(BF16)
        w_arr[pidx, sp] = ws_.astype(BF16)
        idx2d = np.tile(idx_flat.reshape(-1, 16).T, (8, 1))
        percore.append(dict(idx=np.ascontiguousarray(idx2d),
                            rmb=np.ascontiguousarray(rmb_arr.T),
                            w=np.ascontiguousarray(w_arr.T),
                            pos=pos_l[c]))
    return plan, percore


def build_spmm_graph(nc, pools, name, plan, iota_b, qctr):
    f32 = mybir.dt.float32
    bf16 = mybir.dt.bfloat16
    n_pair = plan['n_src'] // 2
    tabs = [nc.dram_tensor(f"{name}_tab{r}", [min(RANGE, n_pair - r * RANGE), 64],
                           f32, kind="ExternalInput")
            for r in range(plan['nR'])]
    idx_d = nc.dram_tensor(f"{name}_idx", [P, plan['total_slots'] // 16],
                           mybir.dt.int16, kind="ExternalInput")
    rmb_d = nc.dram_tensor(f"{name}_rmb", [P, plan['npass']], bf16,
                           kind="ExternalInput")
    w_d = nc.dram_tensor(f"{name}_w", [P, plan['npass']], bf16,
                         kind="ExternalInput")
    out_d = nc.dram_tensor(f"{name}_out", [plan['nblocks'] * B, 64], f32,
                           kind="ExternalOutput")
    sbuf, psum, gpool, selpool, eqpool, resid = pools
    # rmb/w pass tables stay resident in SBUF for the whole launch
    rmb_t = resid.tile([P, plan['npass']], bf16, tag=f"rmb_{name}")
    w_t = resid.tile([P, plan['npass']], bf16, tag=f"w_{name}")
    nc.sync.dma_start(rmb_t[:], rmb_d[:])
    nc.sync.dma_start(w_t[:], w_d[:])
    first_ps, last_ps = plan['first_ps'], plan['last_ps']
    from collections import defaultdict
    byS = defaultdict(list)
    for g in plan['groups']:
        byS[g['S']].append(g)
    for S, glist in sorted(byS.items()):
        # one psum tile accumulates the whole superblock (SBLK blocks of 64
        # rows side by side on partitions 0-63)
        pt = psum.tile([B, SBLK * 64], mybir.dt.float32, tag="ps")
        mm_jobs = []
        for g in glist:
            for job in g['jobs']:
                c0, c1 = job['c0'], job['c1']
                nch = c1 - c0
                np_j = job['p1'] - job['p0']
                s_lo = g['slot0'] + c0 * P
                s_hi = g['slot0'] + c1 * P
                it = gpool.tile([P, (s_hi - s_lo) // 16], mybir.dt.int16,
                                tag="idx")
                nc.sync.dma_start(it[:], idx_d[:, s_lo // 16:s_hi // 16])
                gb = gpool.tile([P, nch * 64], f32, tag="gbuf")
                gb3 = gb[:].rearrange("p (c f) -> p c f", f=64)
                gbb = gb[:].bitcast(bf16).rearrange("p (c f) -> p c f", f=128)
                for s0 in range(0, s_hi - s_lo, SUB):
                    gsub = min(SUB, s_hi - s_lo - s0)
                    nc.gpsimd.dma_gather(
                        out_ap=gb3[:, s0 // P:(s0 + gsub) // P, :],
                        in_ap=tabs[g['r']][:],
                        idxs_ap=it[:, s0 // 16:(s0 + gsub) // 16],
                        num_idxs=gsub, num_idxs_reg=gsub, elem_size=64,
                        queue_num=qctr[0] % NQ)
                    qctr[0] += 1
                # bulk selector build over this job's passes:
                # sel[p, k, d] = (rmb[p, p0+k] == d) * w[p, p0+k]
                eq = eqpool.tile([P, np_j * B], bf16, tag="eq")
                eq3 = eq[:].rearrange("p (c d) -> p c d", d=B)
                sel = selpool.tile([P, np_j * B], bf16, tag="sel")
                sel3 = sel[:].rearrange("p (c d) -> p c d", d=B)
                rt_b = rmb_t[:, job['p0']:job['p1']].rearrange(
                    "p (c u) -> p c u", u=1).broadcast_to([P, np_j, B])
                wt_b = w_t[:, job['p0']:job['p1']].rearrange(
                    "p (c u) -> p c u", u=1).broadcast_to([P, np_j, B])
                io_b = iota_b[:].rearrange("p (u d) -> p u d", u=1).broadcast_to(
                    [P, np_j, B])
                nc.vector.scalar_tensor_tensor(
                    out=eq3, in0=rt_b, scalar=1.0, in1=io_b,
                    op0=mybir.AluOpType.mult, op1=mybir.AluOpType.is_equal)
                nc.vector.scalar_tensor_tensor(
                    out=sel3, in0=eq3, scalar=1.0, in1=wt_b,
                    op0=mybir.AluOpType.mult, op1=mybir.AluOpType.mult)
                mm_jobs.append((sel, gbb, job))
        # matmul pass, block-major: PSUM's 2KB zero-region semantics require
        # each block's accumulation chain to be contiguous (a start=True
        # marks the whole bank pending-zero, wiping other blocks' partials)
        per_block = {}
        for sel, gbb, job in mm_jobs:
            for ci_l, segs in enumerate(job['chunk_passes']):
                for b, par, pi in segs:
                    per_block.setdefault(b, []).append(
                        (pi, sel, gbb, ci_l, par, job['p0']))
        for b in sorted(per_block):
            plist = sorted(per_block[b], key=lambda t: t[0])
            bi = b % SBLK
            for j, (pi, sel, gbb, ci_l, par, p0) in enumerate(plist):
                k = pi - p0
                nc.tensor.matmul(
                    pt[:, bi * 64:(bi + 1) * 64],
                    lhsT=sel[:, k * B:(k + 1) * B],
                    rhs=gbb[:, ci_l, par * 64:(par + 1) * 64],
                    start=(j == 0), stop=(j == len(plist) - 1))
        # copy the finished superblock out of PSUM and store
        nbw = plan['nbw'][S]
        acc = sbuf.tile([B, SBLK * 64], mybir.dt.float32, tag="acc")
        nc.scalar.activation(out=acc[:, :nbw * 64], in_=pt[:, :nbw * 64],
                             func=mybir.ActivationFunctionType.Copy)
        ov = out_d[S * SBLK * B:S * SBLK * B + nbw * B, :].rearrange(
            "(q p) f -> p q f", p=B)
        nc.sync.dma_start(ov, acc[:, :nbw * 64].rearrange("p (q f) -> p q f", f=64))


def build_neff(plans):
    nc = bacc.Bacc("TRN2", target_bir_lowering=False, debug=False,
                   num_devices=NC, num_swdge_queues=NQ)
    with tile.TileContext(nc) as tc:
        with tc.tile_pool(name="sbuf", bufs=3) as sbuf, \
             tc.tile_pool(name="gpool", bufs=8) as gpool, \
             tc.tile_pool(name="selpool", bufs=8) as selpool, \
             tc.tile_pool(name="eqpool", bufs=3) as eqpool, \
             tc.tile_pool(name="resid", bufs=1) as resid, \
             tc.tile_pool(name="psum", bufs=4, space="PSUM") as psum, \
             tc.tile_pool(name="const", bufs=1) as constp:
            iota_i = constp.tile([P, B], mybir.dt.int32)
            nc.gpsimd.iota(iota_i[:], pattern=[[1, B]], base=0, channel_multiplier=0)
            iota_b = constp.tile([P, B], mybir.dt.bfloat16)
            nc.vector.tensor_copy(out=iota_b[:], in_=iota_i[:])
            qctr = [0]
            for name, plan in plans.items():
                build_spmm_graph(nc, (sbuf, psum, gpool, selpool, eqpool, resid),
                                 name, plan, iota_b, qctr)
    nc.compile()
    return nc


def to_pair_table(feat):
    """f32 [n, 64] -> bf16 pair rows bit-cast to f32 [n/2, 64]."""
    xb = feat.astype(BF16).reshape(-1, 128).view(np.uint16)
    return np.ascontiguousarray(xb).view(np.float32)


def split_tab(feat):
    pt = to_pair_table(feat)
    return [np.ascontiguousarray(pt[r * RANGE:min((r + 1) * RANGE, len(pt))])
            for r in range(-(-len(pt) // RANGE))]


def run_launch(nc, plans, percores, tables):
    tabs = {name: split_tab(tables[name]) for name in plans}
    in_maps = []
    for c in range(NC):
        m = {}
        for name in plans:
            pc = percores[name][c]
            m[f"{name}_idx"] = pc['idx']
            m[f"{name}_rmb"] = pc['rmb']
            m[f"{name}_w"] = pc['w']
            for r, t in enumerate(tabs[name]):
                m[f"{name}_tab{r}"] = t
        in_maps.append(m)
    import os
    trace = os.environ.get('KTRACE', '0') == '1'
    res = bass_utils.run_bass_kernel_spmd(nc, in_maps, core_ids=list(range(NC)),
                                          trace=trace)
    if res.exec_time_ns:
        globals()['HW_NS'] = globals().get('HW_NS', 0) + int(res.exec_time_ns)
    outs = {}
    for name, plan in plans.items():
        outs[name] = [res.results[c][f"{name}_out"][percores[name][c]['pos']]
                      for c in range(NC)]
    return outs


def asm_users(parts):
    return np.concatenate([p[:US] for p in parts], 0)

def asm_items(parts):
    return np.concatenate([p[:IS] for p in parts], 0)

def asm_ui(parts):
    u = np.concatenate([p[:US] for p in parts], 0)
    i = np.concatenate([p[US:US + IS] for p in parts], 0)
    return np.concatenate([u, i], 0)

# ---------------- host glue (numpy port of reference) ----------------

def l2n(x):
    return x / np.maximum(np.linalg.norm(x, axis=-1, keepdims=True), EPS)

def mlp_np(x, Wp, bp, Wo, bo):
    h = x @ Wp + bp
    h = np.where(h > 0, h, 0.25 * h).astype(np.float32)
    return l2n(h @ Wo + bo)

def norm_w(row, col, val, n):
    deg = np.bincount(row, weights=val, minlength=n).astype(np.float32)
    dis = np.where(deg > 0, np.where(deg > 0, deg, 1.0) ** -0.5, 0.0).astype(np.float32)
    return (val * dis[row] * dis[col]).astype(np.float32)

_CACHE = {}

def _shard_users(r):
    return r // US, r % US

def _shard_items(r):
    return r // IS, r % IS

def _shard_ui(r):
    isu = r < USER_N
    c = np.where(isu, r // US, (r - USER_N) // IS)
    loc = np.where(isu, r % US, US + (r - USER_N) % IS)
    return c, loc

def _split(rows, cols, ws, shard_fn):
    c, loc = shard_fn(rows)
    out = ([], [], [])
    for cc in range(NC):
        m = c == cc
        out[0].append(loc[m])
        out[1].append(cols[m])
        out[2].append(ws[m])
    return out


def kernel(**inp):
    g = lambda k: np.asarray(inp[k])
    uu_row, uu_col, uu_val = g('uu_row'), g('uu_col'), g('uu_val')
    ii_row, ii_col, ii_val = g('ii_row'), g('ii_col'), g('ii_val')
    ui_u, ui_i, ui_val = g('ui_u'), g('ui_i'), g('ui_val')
    user_emb, item_emb = g('user_emb'), g('item_emb')

    # symmetric ui adjacency
    ui_row = np.concatenate([ui_u, ui_i + USER_N])
    ui_colS = np.concatenate([ui_i + USER_N, ui_u])
    ui_v2 = np.concatenate([ui_val, ui_val])

    w_uu = norm_w(uu_row, uu_col, uu_val, USER_N)
    w_ii = norm_w(ii_row, ii_col, ii_val, ITEM_N)
    w_ui = norm_w(ui_row, ui_colS, ui_v2, N)

    import hashlib
    ck = hashlib.sha1(b''.join(
        a[::131].tobytes() for a in
        (uu_row, uu_col, ii_row, ii_col, ui_u, ui_i))).hexdigest()
    if _CACHE.get('key') != ck:
        _CACHE.clear()
        _CACHE['key'] = ck

    if 'A' not in _CACHE:
        pu, du = plan_graph(*_split(uu_row, uu_col, w_uu, _shard_users), US, USER_N)
        pi, di = plan_graph(*_split(ii_row, ii_col, w_ii, _shard_items), IS, ITEM_N)
        pui, dui = plan_graph(*_split(ui_row, ui_colS, w_ui, _shard_ui), US + IS, N)
        plansA = dict(uu=pu, ii=pi, ui=pui)
        dataA = dict(uu=du, ii=di, ui=dui)
        pmu, dmu = plan_graph(*_split(ui_u, ui_i, ui_val, _shard_users), US, ITEM_N)
        pmi, dmi = plan_graph(*_split(ui_i, ui_u, ui_val, _shard_items), IS, USER_N)
        plansB = dict(mu=pmu, mi=pmi)
        dataB = dict(mu=dmu, mi=dmi)
        _CACHE['A'] = (plansA, dataA, build_neff(plansA))
        _CACHE['B'] = (plansB, dataB, build_neff(plansB))
    plansA, dataA, ncA = _CACHE['A']
    plansB, dataB, ncB = _CACHE['B']

    # zero-degree dest rows: their psum region is never written on device,
    # so the copied-out values are garbage — mask them to the exact 0 the
    # reference's segment_sum produces.
    z_uu = np.bincount(uu_row, minlength=USER_N) == 0
    z_ii = np.bincount(ii_row, minlength=ITEM_N) == 0
    z_ui = np.bincount(ui_row, minlength=N) == 0
    z_mu = np.bincount(ui_u, minlength=USER_N) == 0
    z_mi = np.bincount(ui_i, minlength=ITEM_N) == 0

    # gate (host)
    uu0 = (user_emb * (1 / (1 + np.exp(-(user_emb @ g('gwu') + g('gwub')))))).astype(np.float32)
    ii0 = (item_emb * (1 / (1 + np.exp(-(item_emb @ g('gwi') + g('gwib')))))).astype(np.float32)
    uiE = np.concatenate([user_emb, item_emb], 0)
    all_u, all_i, all_ui = [uu0], [ii0], [uiE]
    uE, iE = uu0, ii0
    for _ in range(2):
        o = run_launch(ncA, plansA, dataA,
                       dict(uu=uE, ii=iE, ui=uiE))
        u0 = asm_users(o['uu'])
        i0 = asm_items(o['ii'])
        ui0 = asm_ui(o['ui'])
        u0[z_uu] = 0.0
        i0[z_ii] = 0.0
        ui0[z_ui] = 0.0
        uE = ((u0 + ui0[:USER_N]) * 0.5).astype(np.float32)
        iE = ((i0 + ui0[USER_N:]) * 0.5).astype(np.float32)
        uiE = np.concatenate([uE, iE], 0)
        all_u.append(l2n(u0).astype(np.float32))
        all_i.append(l2n(i0).astype(np.float32))
        all_ui.append(l2n(ui0).astype(np.float32))
    userEmb = np.mean(np.stack(all_u, 1), 1).astype(np.float32)
    itemEmb = np.mean(np.stack(all_i, 1), 1).astype(np.float32)
    uiEmb = np.mean(np.stack(all_ui, 1), 1).astype(np.float32)
    ui_uE, ui_iE = uiEmb[:USER_N], uiEmb[USER_N:]

    o = run_launch(ncB, plansB, dataB, dict(mu=ui_iE, mi=ui_uE))
    uneigh = asm_users(o['mu'])
    ineigh = asm_items(o['mi'])
    uneigh[z_mu] = 0.0
    ineigh[z_mi] = 0.0

    tu = (np.concatenate([userEmb, ui_uE, uneigh], 1) @ g('meta_u_W') + g('meta_u_b')).astype(np.float32)
    ti = (np.concatenate([itemEmb, ui_iE, ineigh], 1) @ g('meta_i_W') + g('meta_i_b')).astype(np.float32)
    mu1 = mlp_np(tu, g('m0_Wp'), g('m0_bp'), g('m0_Wo'), g('m0_bo')).reshape(-1, D, K)
    mu2 = mlp_np(tu, g('m1_Wp'), g('m1_bp'), g('m1_Wo'), g('m1_bo')).reshape(-1, K, D)
    mi1 = mlp_np(ti, g('m2_Wp'), g('m2_bp'), g('m2_Wo'), g('m2_bo')).reshape(-1, D, K)
    mi2 = mlp_np(ti, g('m3_Wp'), g('m3_bp'), g('m3_Wo'), g('m3_bo')).reshape(-1, K, D)

    def smax(x, ax):
        e = np.exp(x - x.max(axis=ax, keepdims=True))
        return (e / e.sum(axis=ax, keepdims=True)).astype(np.float32)
    lwu1 = smax(mu1 + mu1.mean(0), 1)
    lwu2 = smax(mu2 + mu2.mean(0), 1)
    lwi1 = smax(mi1 + mi1.mean(0), 1)
    lwi2 = smax(mi2 + mi2.mean(0), 1)
    tus = np.einsum('nd,ndk->nk', userEmb, lwu1)
    tus = np.einsum('nk,nkd->nd', tus, lwu2)
    tis = np.einsum('nd,ndk->nk', itemEmb, lwi1)
    tis = np.einsum('nk,nkd->nd', tis, lwi2)
    return np.concatenate([userEmb + tus, itemEmb + tis], 0).astype(np.float32)


# revision 45
# speedup vs baseline: 1.3193x; 1.0390x over previous
"""HGCL forward on 8 Trainium2 NeuronCores.

Strategy: the memory-bound core of this model is 8 SpMMs over ~10M directed
edges (2 GNN layers x 3 graphs + 2 meta aggregations). Each SpMM runs on
device, dest-node-sharded across the 8 cores: edge messages are gathered from
HBM feature tables with dma_gather (256B descriptors carrying bf16 feature
PAIRS bit-cast as f32, round-robined over all 4 SWDGE queues), reduced into
64-row destination blocks with per-chunk one-hot matmuls in bf16 on the
tensor engine (PSUM-accumulated per superblock), and written back dense.

Descriptor count is the bottleneck (~2.26 ns/descriptor at 4 queues), so
bins are packed back-to-back with NO per-bin chunk padding: a 128-slot chunk
may span several (dest-block, parity) bins, and each (chunk, bin) segment
gets its own matmul pass whose per-pass weight column zeroes foreign slots.
One-hot selectors are built in bulk on the vector engine (two bf16
scalar_tensor_tensor ops per job using stride-0 broadcast access patterns)
from rmb/w pass tables resident in SBUF. Cheap dense glue (gating, l2-norms,
means, meta MLPs, softmax/einsum head) runs on host between the three device
launches.
"""
import numpy as np, sys
sys.path.insert(0, '/opt/trn_rl_repo')
import ml_dtypes
import concourse.bacc as bacc
import concourse.tile as tile
import concourse.mybir as mybir
from concourse import bass_utils

USER_N, ITEM_N, D, K = 50000, 80000, 64, 4
N = USER_N + ITEM_N
NC = 8
US, IS = USER_N // NC, ITEM_N // NC   # 6250, 10000 per-core shards
RANGE = 32768        # pair-rows per gather table (= 65536 nodes)
P = 128              # slots per chunk (PE contraction width)
B = 64               # dest rows per block (one-hot width)
SBLK = 8             # blocks of B per superblock (psum tile = [64, SBLK*64])
SUB = 1024           # idxs per dma_gather instruction (HW ucode max)
NQ = 4               # SWDGE queues (ucode MAX_SWDGE_QUEUES)
TCH = 32             # chunks per pipeline job
EPS = 1e-12
BF16 = ml_dtypes.bfloat16

# ---------------- host planning ----------------

def plan_graph(rows_l, cols_l, ws_l, n_dest_local, n_src, wins=None):
    """Bin edges by (superblock S, source range r, dest block b, src parity)
    per core. Within each (S, r) group, bins are packed back-to-back (slot
    count per bin = max count across cores, NO rounding); chunks are 128-slot
    windows over the packed group, and every (chunk, bin) overlap becomes one
    matmul pass. All cores run an identical program."""
    nblocks = -(-n_dest_local // B)
    nS = -(-nblocks // SBLK)
    # signed int16 gather indices + mid-table AP windows: each window is
    # (pair_lo, pair_hi, base); idx = pair - base must fit int16 and no
    # gather instruction may be entirely negative (the ucode drops a
    # trailing run of negative idxs; an all-negative gather hangs)
    n_pair = n_src // 2
    if wins is None:
        wins = [(0, n_pair, RANGE if n_pair > RANGE else 0)]
    win_lo = np.array([w[0] for w in wins], dtype=np.int64)
    win_base = np.array([w[2] for w in wins], dtype=np.int64)
    nR = len(wins)
    nD = nR * 2

    def win_of(pairs):
        return (np.searchsorted(win_lo[1:], pairs, side='right')
                if nR > 1 else np.zeros(len(pairs), dtype=np.int64))

    # --- balance dest rows into blocks so every core's (r, par) bin counts
    # approach the same per-block quota (shrinks the max-across-cores pad) ---
    deg_all = np.zeros((NC, n_dest_local, nD), dtype=np.int32)
    for c in range(NC):
        dim = win_of(cols_l[c] >> 1) * 2 + (cols_l[c] & 1)
        np.add.at(deg_all, (c, rows_l[c].astype(np.int64), dim), 1)
    tgt = deg_all.sum(axis=(0, 1)) / (NC * nblocks)  # shared quota [nD]
    blockof_l, rank_l, pos_l = [], [], []
    for c in range(NC):
        deg = deg_all[c].astype(np.float64)
        order = np.argsort(-deg.sum(1), kind='stable')
        loads = np.zeros((nblocks, nD))
        over = np.zeros(nblocks)                 # current overflow sum
        nin = np.zeros(nblocks, dtype=np.int64)
        blockof = np.empty(n_dest_local, dtype=np.int64)
        rank = np.empty(n_dest_local, dtype=np.int64)
        for row in order:
            d = deg[row]
            inc = np.clip(loads + d - tgt, 0, None).sum(1) - over
            inc[nin >= B] = np.inf
            b = int(np.argmin(inc))
            blockof[row] = b
            rank[row] = nin[b]
            loads[b] += d
            over[b] += inc[b]
            nin[b] += 1
        blockof_l.append(blockof)
        rank_l.append(rank)
        pos_l.append(blockof * B + rank)

    counts = np.zeros((NC, nR, nblocks, 2), dtype=np.int64)
    for c in range(NC):
        b = blockof_l[c][rows_l[c]]
        r = win_of(cols_l[c] >> 1)
        par = cols_l[c] & 1
        np.add.at(counts, (c, r, b, par), 1)
    maxc = counts.max(axis=0)                   # [nR, nblocks, 2]

    groups = []          # per (S, r): slot/chunk/pass layout
    bin_slot = np.full((nR, nblocks, 2), -1, dtype=np.int64)
    pass_of = {}         # (global_chunk, b, par) -> global pass index
    pass_meta = []       # per pass: (S, b, par, chunk_global)
    slot_off = 0
    chunk_off = 0
    npass = 0
    for S in range(nS):
        for r in range(nR):
            bins = []
            g_slot0 = slot_off
            for b in range(S * SBLK, min((S + 1) * SBLK, nblocks)):
                for par in (0, 1):
                    n = int(maxc[r, b, par])
                    if n == 0:
                        continue
                    bin_slot[r, b, par] = slot_off
                    bins.append((b, par, slot_off, n))
                    slot_off += n
            if not bins:
                continue
            g_slots = slot_off - g_slot0
            nch = -(-g_slots // P)
            slot_off = g_slot0 + nch * P        # pad group tail to whole chunks
            # jobs: runs of TCH chunks
            jobs = []
            for j0 in range(0, nch, TCH):
                j1 = min(j0 + TCH, nch)
                jp0 = npass
                chunk_passes = []               # per chunk: [(b, par, pass)]
                for ci in range(j0, j1):
                    c_lo = g_slot0 + ci * P
                    c_hi = c_lo + P
                    segs = []
                    for b, par, s0, n in bins:
                        if s0 < c_hi and s0 + n > c_lo:
                            pass_of[(chunk_off + ci, b, par)] = npass
                            pass_meta.append((S, b, par, chunk_off + ci))
                            segs.append((b, par, npass))
                            npass += 1
                    chunk_passes.append(segs)
                jobs.append(dict(c0=j0, c1=j1, p0=jp0, p1=npass,
                                 chunk_passes=chunk_passes))
            groups.append(dict(S=S, r=r, slot0=g_slot0, nch=nch,
                               chunk0=chunk_off, jobs=jobs))
            chunk_off += nch
    total_slots = slot_off
    total_chunks = chunk_off
    # first/last pass per (S, b) for psum start/stop + written-width per S
    first_ps, last_ps = {}, {}
    nbw = {}
    for pi, (S, b, par, cg) in enumerate(pass_meta):
        if (S, b) not in first_ps:
            first_ps[(S, b)] = pi
        last_ps[(S, b)] = pi
        nbw[S] = max(nbw.get(S, 0), b - S * SBLK + 1)
    plan = dict(nblocks=nblocks, nS=nS, nR=nR, groups=groups,
                total_slots=total_slots, total_chunks=total_chunks,
                npass=npass, first_ps=first_ps, last_ps=last_ps, nbw=nbw,
                n_src=n_src, win_base=[int(x) for x in win_base])

    # sorted pass-key table for vectorized (chunk, b, par) -> pass lookup
    pk = np.array([(cg * nblocks + b) * 2 + par
                   for (S, b, par, cg) in pass_meta], dtype=np.int64)
    pk_order = np.argsort(pk, kind='stable')
    pk_sorted = pk[pk_order]

    # last slot of every gather instruction window: the ucode drops a
    # TRAILING run of negative (signed) indices, so these slots must end up
    # with idx >= 0 on every core (fixed per core by an in-bin slot swap)
    wends = []
    for g2 in groups:
        for job in g2['jobs']:
            s_lo = g2['slot0'] + job['c0'] * P
            s_hi = g2['slot0'] + job['c1'] * P
            for s0 in range(s_lo, s_hi, SUB):
                wends.append(min(s0 + SUB, s_hi) - 1)
    is_wend = np.zeros(total_slots, dtype=bool)
    is_wend[wends] = True

    # ---- per-core data: idx per slot, rmb/w per (pass, slotpos) ----
    percore = []
    for c in range(NC):
        rs, cs_, ws_ = rows_l[c], cols_l[c], ws_l[c]
        b_s = blockof_l[c][rs]
        rk_s = rank_l[c][rs]
        r_s = win_of(cs_ >> 1)
        par_s = (cs_ & 1).astype(np.int64)
        # position within bin
        key = (r_s * nblocks + b_s) * 2 + par_s
        so = np.argsort(key, kind='stable')
        pos = np.zeros(len(rs), dtype=np.int64)
        _, fi, ct = np.unique(key[so], return_index=True, return_counts=True)
        for f0, c0 in zip(fi, ct):
            pos[so[f0:f0 + c0]] = np.arange(c0)
        base = bin_slot[r_s, b_s, par_s]
        slot = base + pos
        idx_flat = np.zeros(total_slots, dtype=np.int16)
        idx_flat[slot] = ((cs_ >> 1) - win_base[r_s]).astype(np.int16)
        if win_base.any():
            edge_of_slot = np.full(total_slots, -1, dtype=np.int64)
            edge_of_slot[slot] = np.arange(len(rs))
            for we in wends:
                if idx_flat[we] >= 0:
                    continue
                e = edge_of_slot[we]
                lo = int(bin_slot[r_s[e], b_s[e], par_s[e]])
                hi = lo + int(maxc[r_s[e], b_s[e], par_s[e]])
                seg_ok = (idx_flat[lo:hi] >= 0) & ~is_wend[lo:hi]
                cand = np.nonzero(seg_ok)[0]
                if len(cand) == 0:
                    cand = np.nonzero(idx_flat[lo:hi] >= 0)[0]
                    if len(cand) == 0:
                        continue
                cs2 = lo + int(cand[0])
                if cs2 == we:
                    continue
                e2 = edge_of_slot[cs2]
                idx_flat[we], idx_flat[cs2] = idx_flat[cs2], idx_flat[we]
                slot[e] = cs2
                if e2 >= 0:
                    slot[e2] = we
                edge_of_slot[we], edge_of_slot[cs2] = e2, e
        cg = slot // P
        sp = slot % P
        ek = (cg * nblocks + b_s) * 2 + par_s
        pidx = pk_order[np.searchsorted(pk_sorted, ek)]
        rmb_arr = np.zeros((npass, P), dtype=BF16)
        w_arr = np.zeros((npass, P), dtype=BF16)
        rmb_arr[pidx, sp] = rk_s.astype(BF16)
        w_arr[pidx, sp] = ws_.astype(BF16)
        idx2d = np.tile(idx_flat.reshape(-1, 16).T, (8, 1))
        percore.append(dict(idx=np.ascontiguousarray(idx2d),
                            rmb=np.ascontiguousarray(rmb_arr.T),
                            w=np.ascontiguousarray(w_arr.T),
                            pos=pos_l[c]))
    return plan, percore


def build_spmm_graph(nc, pools, name, plan, iota_b, qctr):
    f32 = mybir.dt.float32
    bf16 = mybir.dt.bfloat16
    n_pair = plan['n_src'] // 2
    tab_d = nc.dram_tensor(f"{name}_tab0", [n_pair, 64], f32,
                           kind="ExternalInput")
    # signed idx windows: AP base at each window's `base` pair row
    tab_aps = [tab_d[b0:, :] if b0 else tab_d[:] for b0 in plan['win_base']]
    idx_d = nc.dram_tensor(f"{name}_idx", [P, plan['total_slots'] // 16],
                           mybir.dt.int16, kind="ExternalInput")
    rmb_d = nc.dram_tensor(f"{name}_rmb", [P, plan['npass']], bf16,
                           kind="ExternalInput")
    w_d = nc.dram_tensor(f"{name}_w", [P, plan['npass']], bf16,
                         kind="ExternalInput")
    out_d = nc.dram_tensor(f"{name}_out", [plan['nblocks'] * B, 64], f32,
                           kind="ExternalOutput")
    sbuf, psum, gpool, selpool, eqpool, resid = pools
    # rmb/w pass tables stay resident in SBUF for the whole launch
    rmb_t = resid.tile([P, plan['npass']], bf16, tag=f"rmb_{name}")
    w_t = resid.tile([P, plan['npass']], bf16, tag=f"w_{name}")
    nc.sync.dma_start(rmb_t[:], rmb_d[:])
    nc.sync.dma_start(w_t[:], w_d[:])
    first_ps, last_ps = plan['first_ps'], plan['last_ps']
    from collections import defaultdict
    byS = defaultdict(list)
    for g in plan['groups']:
        byS[g['S']].append(g)
    for S, glist in sorted(byS.items()):
        # one psum tile accumulates the whole superblock (SBLK blocks of 64
        # rows side by side on partitions 0-63)
        pt = psum.tile([B, SBLK * 64], mybir.dt.float32, tag="ps")
        mm_jobs = []
        for g in glist:
            for job in g['jobs']:
                c0, c1 = job['c0'], job['c1']
                nch = c1 - c0
                np_j = job['p1'] - job['p0']
                s_lo = g['slot0'] + c0 * P
                s_hi = g['slot0'] + c1 * P
                it = gpool.tile([P, (s_hi - s_lo) // 16], mybir.dt.int16,
                                tag="idx")
                nc.sync.dma_start(it[:], idx_d[:, s_lo // 16:s_hi // 16])
                gb = gpool.tile([P, nch * 64], f32, tag="gbuf")
                gb3 = gb[:].rearrange("p (c f) -> p c f", f=64)
                gbb = gb[:].bitcast(bf16).rearrange("p (c f) -> p c f", f=128)
                for s0 in range(0, s_hi - s_lo, SUB):
                    gsub = min(SUB, s_hi - s_lo - s0)
                    nc.gpsimd.dma_gather(
                        out_ap=gb3[:, s0 // P:(s0 + gsub) // P, :],
                        in_ap=tab_aps[g['r']],
                        idxs_ap=it[:, s0 // 16:(s0 + gsub) // 16],
                        num_idxs=gsub, num_idxs_reg=gsub, elem_size=64,
                        queue_num=qctr[0] % NQ)
                    qctr[0] += 1
                # bulk selector build over this job's passes:
                # sel[p, k, d] = (rmb[p, p0+k] == d) * w[p, p0+k]
                eq = eqpool.tile([P, np_j * B], bf16, tag="eq")
                eq3 = eq[:].rearrange("p (c d) -> p c d", d=B)
                sel = selpool.tile([P, np_j * B], bf16, tag="sel")
                sel3 = sel[:].rearrange("p (c d) -> p c d", d=B)
                rt_b = rmb_t[:, job['p0']:job['p1']].rearrange(
                    "p (c u) -> p c u", u=1).broadcast_to([P, np_j, B])
                wt_b = w_t[:, job['p0']:job['p1']].rearrange(
                    "p (c u) -> p c u", u=1).broadcast_to([P, np_j, B])
                io_b = iota_b[:].rearrange("p (u d) -> p u d", u=1).broadcast_to(
                    [P, np_j, B])
                nc.vector.scalar_tensor_tensor(
                    out=eq3, in0=rt_b, scalar=1.0, in1=io_b,
                    op0=mybir.AluOpType.mult, op1=mybir.AluOpType.is_equal)
                nc.vector.scalar_tensor_tensor(
                    out=sel3, in0=eq3, scalar=1.0, in1=wt_b,
                    op0=mybir.AluOpType.mult, op1=mybir.AluOpType.mult)
                mm_jobs.append((sel, gbb, job))
        # matmul pass, block-major: PSUM's 2KB zero-region semantics require
        # each block's accumulation chain to be contiguous (a start=True
        # marks the whole bank pending-zero, wiping other blocks' partials)
        per_block = {}
        for sel, gbb, job in mm_jobs:
            for ci_l, segs in enumerate(job['chunk_passes']):
                for b, par, pi in segs:
                    per_block.setdefault(b, []).append(
                        (pi, sel, gbb, ci_l, par, job['p0']))
        for b in sorted(per_block):
            plist = sorted(per_block[b], key=lambda t: t[0])
            bi = b % SBLK
            for j, (pi, sel, gbb, ci_l, par, p0) in enumerate(plist):
                k = pi - p0
                nc.tensor.matmul(
                    pt[:, bi * 64:(bi + 1) * 64],
                    lhsT=sel[:, k * B:(k + 1) * B],
                    rhs=gbb[:, ci_l, par * 64:(par + 1) * 64],
                    start=(j == 0), stop=(j == len(plist) - 1))
        # copy the finished superblock out of PSUM and store
        nbw = plan['nbw'][S]
        acc = sbuf.tile([B, SBLK * 64], mybir.dt.float32, tag="acc")
        nc.scalar.activation(out=acc[:, :nbw * 64], in_=pt[:, :nbw * 64],
                             func=mybir.ActivationFunctionType.Copy)
        ov = out_d[S * SBLK * B:S * SBLK * B + nbw * B, :].rearrange(
            "(q p) f -> p q f", p=B)
        nc.sync.dma_start(ov, acc[:, :nbw * 64].rearrange("p (q f) -> p q f", f=64))


def build_neff(plans):
    nc = bacc.Bacc("TRN2", target_bir_lowering=False, debug=False,
                   num_devices=NC, num_swdge_queues=NQ)
    with tile.TileContext(nc) as tc:
        with tc.tile_pool(name="sbuf", bufs=3) as sbuf, \
             tc.tile_pool(name="gpool", bufs=8) as gpool, \
             tc.tile_pool(name="selpool", bufs=8) as selpool, \
             tc.tile_pool(name="eqpool", bufs=3) as eqpool, \
             tc.tile_pool(name="resid", bufs=1) as resid, \
             tc.tile_pool(name="psum", bufs=4, space="PSUM") as psum, \
             tc.tile_pool(name="const", bufs=1) as constp:
            iota_i = constp.tile([P, B], mybir.dt.int32)
            nc.gpsimd.iota(iota_i[:], pattern=[[1, B]], base=0, channel_multiplier=0)
            iota_b = constp.tile([P, B], mybir.dt.bfloat16)
            nc.vector.tensor_copy(out=iota_b[:], in_=iota_i[:])
            qctr = [0]
            for name, plan in plans.items():
                build_spmm_graph(nc, (sbuf, psum, gpool, selpool, eqpool, resid),
                                 name, plan, iota_b, qctr)
    nc.compile()
    return nc


def to_pair_table(feat):
    """f32 [n, 64] -> bf16 pair rows bit-cast to f32 [n/2, 64]."""
    xb = feat.astype(BF16).reshape(-1, 128).view(np.uint16)
    return np.ascontiguousarray(xb).view(np.float32)


def run_launch(nc, plans, percores, tables):
    tabs = {name: to_pair_table(tables[name]) for name in plans}
    in_maps = []
    for c in range(NC):
        m = {}
        for name in plans:
            pc = percores[name][c]
            m[f"{name}_idx"] = pc['idx']
            m[f"{name}_rmb"] = pc['rmb']
            m[f"{name}_w"] = pc['w']
            m[f"{name}_tab0"] = tabs[name]
        in_maps.append(m)
    import os
    trace = os.environ.get('KTRACE', '0') == '1'
    res = bass_utils.run_bass_kernel_spmd(nc, in_maps, core_ids=list(range(NC)),
                                          trace=trace)
    if res.exec_time_ns:
        globals()['HW_NS'] = globals().get('HW_NS', 0) + int(res.exec_time_ns)
    outs = {}
    for name, plan in plans.items():
        outs[name] = [res.results[c][f"{name}_out"][percores[name][c]['pos']]
                      for c in range(NC)]
    return outs


def asm_users(parts):
    return np.concatenate([p[:US] for p in parts], 0)

def asm_items(parts):
    return np.concatenate([p[:IS] for p in parts], 0)

def asm_ui(parts):
    u = np.concatenate([p[:US] for p in parts], 0)
    i = np.concatenate([p[US:US + IS] for p in parts], 0)
    return np.concatenate([u, i], 0)

# ---------------- host glue (numpy port of reference) ----------------

def l2n(x):
    return x / np.maximum(np.linalg.norm(x, axis=-1, keepdims=True), EPS)

def mlp_np(x, Wp, bp, Wo, bo):
    h = x @ Wp + bp
    h = np.where(h > 0, h, 0.25 * h).astype(np.float32)
    return l2n(h @ Wo + bo)

def norm_w(row, col, val, n):
    deg = np.bincount(row, weights=val, minlength=n).astype(np.float32)
    dis = np.where(deg > 0, np.where(deg > 0, deg, 1.0) ** -0.5, 0.0).astype(np.float32)
    return (val * dis[row] * dis[col]).astype(np.float32)

_CACHE = {}

def _shard_users(r):
    return r // US, r % US

def _shard_items(r):
    return r // IS, r % IS

def _shard_ui(r):
    isu = r < USER_N
    c = np.where(isu, r // US, (r - USER_N) // IS)
    loc = np.where(isu, r % US, US + (r - USER_N) % IS)
    return c, loc

def _split(rows, cols, ws, shard_fn):
    c, loc = shard_fn(rows)
    out = ([], [], [])
    for cc in range(NC):
        m = c == cc
        out[0].append(loc[m])
        out[1].append(cols[m])
        out[2].append(ws[m])
    return out


def kernel(**inp):
    g = lambda k: np.asarray(inp[k])
    uu_row, uu_col, uu_val = g('uu_row'), g('uu_col'), g('uu_val')
    ii_row, ii_col, ii_val = g('ii_row'), g('ii_col'), g('ii_val')
    ui_u, ui_i, ui_val = g('ui_u'), g('ui_i'), g('ui_val')
    user_emb, item_emb = g('user_emb'), g('item_emb')

    # symmetric ui adjacency
    ui_row = np.concatenate([ui_u, ui_i + USER_N])
    ui_colS = np.concatenate([ui_i + USER_N, ui_u])
    ui_v2 = np.concatenate([ui_val, ui_val])

    w_uu = norm_w(uu_row, uu_col, uu_val, USER_N)
    w_ii = norm_w(ii_row, ii_col, ii_val, ITEM_N)
    w_ui = norm_w(ui_row, ui_colS, ui_v2, N)

    import hashlib
    ck = hashlib.sha1(b''.join(
        a[::131].tobytes() for a in
        (uu_row, uu_col, ii_row, ii_col, ui_u, ui_i))).hexdigest()
    if _CACHE.get('key') != ck:
        _CACHE.clear()
        _CACHE['key'] = ck

    if 'A' not in _CACHE:
        pu, du = plan_graph(*_split(uu_row, uu_col, w_uu, _shard_users), US, USER_N)
        pi, di = plan_graph(*_split(ii_row, ii_col, w_ii, _shard_items), IS, ITEM_N)
        # ui sources split cleanly by dest type (i-dest rows source users,
        # u-dest rows source items): window each side so no gather can be
        # all-negative
        UH = USER_N // 2
        ui_wins = [(0, UH, 0), (UH, N // 2, UH + RANGE)]
        pui, dui = plan_graph(*_split(ui_row, ui_colS, w_ui, _shard_ui),
                              US + IS, N, wins=ui_wins)
        plansA = dict(uu=pu, ii=pi, ui=pui)
        dataA = dict(uu=du, ii=di, ui=dui)
        pmu, dmu = plan_graph(*_split(ui_u, ui_i, ui_val, _shard_users), US, ITEM_N)
        pmi, dmi = plan_graph(*_split(ui_i, ui_u, ui_val, _shard_items), IS, USER_N)
        plansB = dict(mu=pmu, mi=pmi)
        dataB = dict(mu=dmu, mi=dmi)
        _CACHE['A'] = (plansA, dataA, build_neff(plansA))
        _CACHE['B'] = (plansB, dataB, build_neff(plansB))
    plansA, dataA, ncA = _CACHE['A']
    plansB, dataB, ncB = _CACHE['B']

    # zero-degree dest rows: their psum region is never written on device,
    # so the copied-out values are garbage — mask them to the exact 0 the
    # reference's segment_sum produces.
    z_uu = np.bincount(uu_row, minlength=USER_N) == 0
    z_ii = np.bincount(ii_row, minlength=ITEM_N) == 0
    z_ui = np.bincount(ui_row, minlength=N) == 0
    z_mu = np.bincount(ui_u, minlength=USER_N) == 0
    z_mi = np.bincount(ui_i, minlength=ITEM_N) == 0

    # gate (host)
    uu0 = (user_emb * (1 / (1 + np.exp(-(user_emb @ g('gwu') + g('gwub')))))).astype(np.float32)
    ii0 = (item_emb * (1 / (1 + np.exp(-(item_emb @ g('gwi') + g('gwib')))))).astype(np.float32)
    uiE = np.concatenate([user_emb, item_emb], 0)
    all_u, all_i, all_ui = [uu0], [ii0], [uiE]
    uE, iE = uu0, ii0
    for _ in range(2):
        o = run_launch(ncA, plansA, dataA,
                       dict(uu=uE, ii=iE, ui=uiE))
        u0 = asm_users(o['uu'])
        i0 = asm_items(o['ii'])
        ui0 = asm_ui(o['ui'])
        u0[z_uu] = 0.0
        i0[z_ii] = 0.0
        ui0[z_ui] = 0.0
        uE = ((u0 + ui0[:USER_N]) * 0.5).astype(np.float32)
        iE = ((i0 + ui0[USER_N:]) * 0.5).astype(np.float32)
        uiE = np.concatenate([uE, iE], 0)
        all_u.append(l2n(u0).astype(np.float32))
        all_i.append(l2n(i0).astype(np.float32))
        all_ui.append(l2n(ui0).astype(np.float32))
    userEmb = np.mean(np.stack(all_u, 1), 1).astype(np.float32)
    itemEmb = np.mean(np.stack(all_i, 1), 1).astype(np.float32)
    uiEmb = np.mean(np.stack(all_ui, 1), 1).astype(np.float32)
    ui_uE, ui_iE = uiEmb[:USER_N], uiEmb[USER_N:]

    o = run_launch(ncB, plansB, dataB, dict(mu=ui_iE, mi=ui_uE))
    uneigh = asm_users(o['mu'])
    ineigh = asm_items(o['mi'])
    uneigh[z_mu] = 0.0
    ineigh[z_mi] = 0.0

    tu = (np.concatenate([userEmb, ui_uE, uneigh], 1) @ g('meta_u_W') + g('meta_u_b')).astype(np.float32)
    ti = (np.concatenate([itemEmb, ui_iE, ineigh], 1) @ g('meta_i_W') + g('meta_i_b')).astype(np.float32)
    mu1 = mlp_np(tu, g('m0_Wp'), g('m0_bp'), g('m0_Wo'), g('m0_bo')).reshape(-1, D, K)
    mu2 = mlp_np(tu, g('m1_Wp'), g('m1_bp'), g('m1_Wo'), g('m1_bo')).reshape(-1, K, D)
    mi1 = mlp_np(ti, g('m2_Wp'), g('m2_bp'), g('m2_Wo'), g('m2_bo')).reshape(-1, D, K)
    mi2 = mlp_np(ti, g('m3_Wp'), g('m3_bp'), g('m3_Wo'), g('m3_bo')).reshape(-1, K, D)

    def smax(x, ax):
        e = np.exp(x - x.max(axis=ax, keepdims=True))
        return (e / e.sum(axis=ax, keepdims=True)).astype(np.float32)
    lwu1 = smax(mu1 + mu1.mean(0), 1)
    lwu2 = smax(mu2 + mu2.mean(0), 1)
    lwi1 = smax(mi1 + mi1.mean(0), 1)
    lwi2 = smax(mi2 + mi2.mean(0), 1)
    tus = np.einsum('nd,ndk->nk', userEmb, lwu1)
    tus = np.einsum('nk,nkd->nd', tus, lwu2)
    tis = np.einsum('nd,ndk->nk', itemEmb, lwi1)
    tis = np.einsum('nk,nkd->nd', tis, lwi2)
    return np.concatenate([userEmb + tus, itemEmb + tis], 0).astype(np.float32)
